# revision 34
# baseline (speedup 1.0000x reference)
"""Causal scaled-dot attention (with QKV projections) on 8 TRN2 NeuronCores.

Sharding: data-parallel over (batch, query-half).  core = (b = c//2, h = c%2).
Each core owns 16 q-tiles of 128 rows of one batch, interleaved so that every
core sees the identical causal structure: local tile t maps to global tile
g = 4*(t//2) + 2*h + (t%2), giving W(t) = t//2 + 1 key-chunks of 512 for all
cores (SPMD-uniform graph), and the within-diagonal mask offset m = g%4 =
2*h + (t%2) is supplied as per-core mask *data*.

Per-core on-chip pipeline (all matmuls bf16, f32 PSUM accumulation):
  qT/kT projections in transposed layout (bias per-partition on ACT),
  v projection in natural layout (bias via a K=1 ones-matmul row),
  scores = qT.T @ kT  -> additive -1e9 causal mask on the diagonal chunk ->
  exp on ACT (scale=1/sqrt(H), row-sums via accum_out) ->
  xbar DMA-transpose of the bf16 exp tiles -> AV matmul accumulated in PSUM ->
  context scaled by 1/rowsum;  attention output = exp * 1/rowsum (f32).
The strictly-masked key-chunks are never computed nor written: output DRAM is
pre-zeroed by the runtime (run_bass_kernel_spmd contract).
"""

import os
import sys

sys.path.insert(0, "/opt/trn_rl_repo")
os.environ.setdefault("MYCRO_LOCAL_CACHE", "1")
os.environ.setdefault("JAX_PLATFORMS", "")

import numpy as np
import ml_dtypes

import concourse.bass as bass  # noqa: F401  (engine types referenced via nc)
import concourse.mybir as mybir
import concourse.tile as tile
from concourse import bacc
from concourse.bass_utils import run_bass_kernel_spmd

BF16 = ml_dtypes.bfloat16
B, S, H = 4, 4096, 512
NCORES = 8
ROWS = S // 2          # 2048 query rows per core
NT = ROWS // 128       # 16 q-tiles per core
SCALE = 1.0 / float(np.sqrt(H))
NEG = -1.0e9

F32 = mybir.dt.float32
BF = mybir.dt.bfloat16


def _build_body(ctx, tc, p):
    nc = tc.nc
    Exp = mybir.ActivationFunctionType.Exp
    Ident = mybir.ActivationFunctionType.Identity
    Copy = mybir.ActivationFunctionType.Copy
    AX = mybir.AxisListType.X

    consts = ctx.enter_context(tc.tile_pool(name="consts", bufs=1))
    resident = ctx.enter_context(tc.tile_pool(name="resident", bufs=1))
    stage = ctx.enter_context(tc.tile_pool(name="stage", bufs=8))
    pspool = ctx.enter_context(tc.tile_pool(name="ps", bufs=5, space="PSUM"))
    psctx = ctx.enter_context(tc.tile_pool(name="psctx", bufs=3, space="PSUM"))
    work = ctx.enter_context(tc.tile_pool(name="work", bufs=2))
    ptp = ctx.enter_context(tc.tile_pool(name="ptp", bufs=2))

    ldst = nc.scalar      # HWDGE ring for loads + stores (ACT sequencer)
    tring = nc.sync       # HWDGE ring dedicated to xbar transposes (SP sequencer)

    # ---- constants (SWDGE; small) ---------------------------------------
    w_sb = {}
    wq_all = consts.tile([128, 4, H], BF, tag="wq_all")
    w_sb["wq"] = [wq_all[:, c, :] for c in range(4)]
    for wname in ("wk", "wv"):
        tiles = [consts.tile([128, H], BF, tag=f"{wname}{c}", name=f"{wname}{c}")
                 for c in range(4)]
        w_sb[wname] = tiles
    bq_sb = consts.tile([128, 4], F32, tag="bq")
    bk_sb = consts.tile([128, 4], F32, tag="bk")
    bv_sb = consts.tile([128, H], F32, tag="bvf")
    am_sb = consts.tile([128, 1024], F32, tag="am")
    ident = consts.tile([128, 128], BF, tag="ident")
    ldst.dma_start(out=wq_all[:], in_=p["wq"].rearrange("(c q) h -> q c h", q=128))
    nc.gpsimd.dma_start(out=bq_sb[:], in_=p["bq"][:, :])
    nc.gpsimd.dma_start(out=am_sb[:], in_=p["amask"][:, :])
    for c in range(4):
        nc.gpsimd.dma_start(out=w_sb["wk"][c][:], in_=p["wk"][c * 128:(c + 1) * 128, :])
    nc.gpsimd.dma_start(out=bk_sb[:], in_=p["bk"][:, :])
    for c in range(4):
        nc.gpsimd.dma_start(out=w_sb["wv"][c][:], in_=p["wv"][c * 128:(c + 1) * 128, :])
    nc.gpsimd.dma_start(out=bv_sb[:], in_=p["bv"][:, :])
    from concourse.masks import make_identity
    make_identity(nc, ident[:])

    # ---- resident tensors and quarter-granular staging ------------------
    qT = [resident.tile([128, ROWS], BF, tag=f"qT{c}", name=f"qT{c}") for c in range(4)]
    kT = [resident.tile([128, S], BF, tag=f"kT{c}", name=f"kT{c}") for c in range(4)]
    v_sb = [resident.tile([128, 512], BF, tag=f"v{i}", name=f"v{i}") for i in range(32)]

    xq_q = {}   # quarter -> 4 tiles [128, 512]
    xk_q = {}   # quarter -> 4 tiles [128, 1024]
    xv_q = {}   # quarter -> 4 tiles [128, 1024]

    def load_xq(q):
        eng = tring if q == 0 else ldst
        ts_ = [stage.tile([128, 512], BF, tag="stgq", name="stgq", bufs=4) for _ in range(4)]
        for c in range(4):
            eng.dma_start(out=ts_[c][:],
                          in_=p["xq"][c * 128:(c + 1) * 128, q * 512:(q + 1) * 512])
        xq_q[q] = ts_

    def load_xk(q):
        eng = tring if q == 0 else ldst
        ts_ = [stage.tile([128, 1024], BF, tag="stgk", name="stgk") for _ in range(4)]
        for c in range(4):
            eng.dma_start(out=ts_[c][:],
                          in_=p["xk"][c * 128:(c + 1) * 128, q * 1024:(q + 1) * 1024])
        xk_q[q] = ts_

    def load_xv(q):
        ts_ = [stage.tile([128, 1024], BF, tag="stgv", name="stgv") for _ in range(4)]
        for c in range(4):
            ldst.dma_start(out=ts_[c][:],
                           in_=p["xv"][c * 128:(c + 1) * 128, q * 1024:(q + 1) * 1024])
        xv_q[q] = ts_

    def proj_qT(icc):
        for hc in range(4):
            ps = pspool.tile([128, 512], F32, tag="ps")
            for pc in range(4):
                nc.tensor.matmul(
                    ps[:], lhsT=w_sb["wq"][pc][:, hc * 128:(hc + 1) * 128],
                    rhs=xq_q[icc][pc][:], start=(pc == 0), stop=(pc == 3),
                )
            nc.vector.tensor_scalar_add(
                qT[hc][:, icc * 512:(icc + 1) * 512], ps[:], bq_sb[:, hc:hc + 1]
            )

    def proj_kT(sc):
        q, off = sc // 2, (sc % 2) * 512
        for hc in range(4):
            ps = pspool.tile([128, 512], F32, tag="ps")
            for pc in range(4):
                nc.tensor.matmul(
                    ps[:], lhsT=w_sb["wk"][pc][:, hc * 128:(hc + 1) * 128],
                    rhs=xk_q[q][pc][:, off:off + 512], start=(pc == 0), stop=(pc == 3),
                )
            nc.vector.tensor_scalar_add(
                kT[hc][:, sc * 512:(sc + 1) * 512], ps[:], bk_sb[:, hc:hc + 1]
            )

    def proj_v(sc):
        q, off = sc // 8, (sc % 8) * 128
        ps = pspool.tile([128, 512], F32, tag="ps")
        for pc in range(4):
            nc.tensor.matmul(
                ps[:], lhsT=xv_q[q][pc][:, off:off + 128], rhs=w_sb["wv"][pc][:],
                start=(pc == 0), stop=(pc == 3),
            )
        nc.vector.tensor_copy(v_sb[sc][:], ps[:])

    # incremental emission state
    done_qT = [False] * 4
    done_kT = [False] * 8
    done_v = [False] * 4     # per xv quarter (8 v tiles at a time)

    def need_qT(icc):
        if not done_qT[icc]:
            load_xq(icc)
            proj_qT(icc)
            done_qT[icc] = True

    def need_kT(sc):
        if not done_kT[sc]:
            if sc % 2 == 0 and sc // 2 not in xk_q:
                load_xk(sc // 2)
            proj_kT(sc)
            done_kT[sc] = True

    def need_v(q):
        if not done_v[q]:
            load_xv(q)
            for sc in range(8 * q, 8 * q + 8):
                proj_v(sc)
            done_v[q] = True

    # prologue: enough for tiles 0..1
    need_qT(0)
    need_kT(0)
    need_v(0)

    # ---- attention main loop (AV/output stage software-pipelined by 1) ---
    tile_state = {}

    def emit_scores(t):
        W = t // 2 + 1           # number of 512-wide key chunks
        j = t % 2                # mask slice; true offset m is in the mask data
        nd = 2 + 2 * j           # diagonal-chunk 128-subchunks worth computing
        dw = nd * 128            # diagonal-chunk computed width (cols beyond are masked)

        ebf = work.tile([128, S], BF, tag="ebf", bufs=4)
        lc = work.tile([128, 8], F32, tag="lc", bufs=3)
        ch1 = (W + 1) // 2
        pts = []
        for kb in range(W):
            cw = 512 if kb < W - 1 else dw
            ps = pspool.tile([128, 512], F32, tag="ps")
            for pc in range(4):
                nc.tensor.matmul(
                    ps[:, 0:cw], lhsT=qT[pc][:, t * 128:(t + 1) * 128],
                    rhs=kT[pc][:, kb * 512:kb * 512 + cw],
                    start=(pc == 0), stop=(pc == 3),
                )
            if kb == W - 1:
                nc.vector.tensor_add(ps[:, 0:cw], ps[:, 0:cw],
                                     am_sb[:, j * 512:j * 512 + cw])
            nc.scalar.activation(
                out=ebf[:, kb * 512:kb * 512 + cw], in_=ps[:, 0:cw],
                func=Exp, scale=SCALE, accum_out=lc[:, kb:kb + 1],
            )
            # issue each half-strip xbar transpose as soon as its exps are done
            for (lo, hi) in ((0, ch1), (ch1, W)):
                if hi > lo and kb == hi - 1:
                    w_cols = (hi - lo - 1) * 512 + (512 if hi < W else dw)
                    pt = ptp.tile([128, 2048], BF, tag="pt", bufs=6)
                    if W == 1:
                        # tiny final tiles: transpose on the PE itself so the
                        # AV matmuls never wait on a DMA semaphore
                        pst = pspool.tile([128, 512], BF, tag="ps", name="pst")
                        for c in range(nd):
                            nc.tensor.transpose(
                                pst[:, c * 128:(c + 1) * 128],
                                ebf[:, c * 128:(c + 1) * 128], ident[:],
                            )
                        nc.scalar.activation(
                            out=pt[:, 0:w_cols], in_=pst[:, 0:w_cols], func=Copy)
                    else:
                        tring.dma_start_transpose(
                            out=pt[:, 0:w_cols].rearrange(
                                "q (c f) -> q c f", f=128),
                            in_=ebf[:, lo * 512:lo * 512 + w_cols],
                        )
                    pts.append((lo, hi, pt))
        tile_state[t] = (ebf, lc, pts)

    def emit_outputs(t):
        W = t // 2 + 1
        j = t % 2
        nd = 2 + 2 * j
        nmm = (W - 1) * 4 + nd
        ebf, lc, pts = tile_state.pop(t)

        ps_ctx = psctx.tile([128, 512], F32, tag="psctx")
        k = 0
        for (lo, hi, pt) in pts:
            for kb in range(lo, hi):
                nsub = 4 if kb < W - 1 else nd
                for c in range(nsub):
                    cc = kb * 4 + c
                    nc.tensor.matmul(
                        ps_ctx[:],
                        lhsT=pt[:, (cc - lo * 4) * 128:(cc - lo * 4 + 1) * 128],
                        rhs=v_sb[cc][:], start=(k == 0), stop=(k == nmm - 1),
                    )
                    k += 1

        ls = work.tile([128, 1], F32, tag="ls")
        nc.vector.reduce_sum(ls[:], lc[:, 0:W], axis=AX)
        rr = work.tile([128, 1], F32, tag="rr")
        nc.vector.reciprocal(rr[:], ls[:])

        tw = (W - 1) * 512 + nd * 128
        halves = [(0, min((W + 1) // 2 * 512, tw)), ((W + 1) // 2 * 512, tw)]
        for lo, hi in halves:
            if hi <= lo:
                continue
            outf = work.tile([128, 2048], BF, tag="outf", bufs=3)
            nc.vector.tensor_scalar_mul(outf[:, 0:hi - lo], ebf[:, lo:hi], rr[:])
            nc.gpsimd.dma_start(
                out=p["attn"][t * 128:(t + 1) * 128, lo:hi],
                in_=outf[:, 0:hi - lo],
            )
        cs = work.tile([128, 512], F32, tag="cs")
        nc.scalar.activation(out=cs[:], in_=ps_ctx[:], func=Copy, scale=rr[:])
        nc.vector.tensor_add(cs[:], cs[:], bv_sb[:])
        nc.gpsimd.dma_start(out=p["ctx"][t * 128:(t + 1) * 128, :], in_=cs[:])

    order = list(range(2, NT)) + [0, 1]
    for pos, t in enumerate(order):
        W = t // 2 + 1
        nd = 2 + 2 * (t % 2)
        need_qT(t // 4)
        for sc in range(W):
            need_kT(sc)
        for q in range((4 * (W - 1) + nd + 7) // 8):
            need_v(q)
        if pos + 2 < NT:
            tn = order[pos + 2]
            need_qT(tn // 4)
            need_kT(tn // 2)
            need_v(min((4 * (tn // 2) + 4 + 7) // 8, 3))
        if pos > 1:
            emit_outputs(order[pos - 2])
        emit_scores(t)
    emit_outputs(order[-2])
    emit_outputs(order[-1])


def build_graph():
    nc = bacc.Bacc("TRN2", target_bir_lowering=False, debug=False, num_devices=NCORES)
    p = {}
    p["xq"] = nc.declare_dram_parameter("xq", [H, ROWS], BF, isOutput=False)
    p["xk"] = nc.declare_dram_parameter("xk", [H, S], BF, isOutput=False)
    p["xv"] = nc.declare_dram_parameter("xv", [H, S], BF, isOutput=False)
    for w in ("wq", "wk", "wv"):
        p[w] = nc.declare_dram_parameter(w, [H, H], BF, isOutput=False)
    p["bq"] = nc.declare_dram_parameter("bq", [128, 4], F32, isOutput=False)
    p["bk"] = nc.declare_dram_parameter("bk", [128, 4], F32, isOutput=False)
    p["bv"] = nc.declare_dram_parameter("bv", [128, H], F32, isOutput=False)
    p["amask"] = nc.declare_dram_parameter("amask", [128, 1024], F32, isOutput=False)
    p["attn"] = nc.declare_dram_parameter("attn", [ROWS, S], F32, isOutput=True)
    p["ctx"] = nc.declare_dram_parameter("ctx", [ROWS, H], F32, isOutput=True)

    import contextlib

    with contextlib.ExitStack() as ctx:
        tc = ctx.enter_context(tile.TileContext(nc))
        _build_body(ctx, tc, p)
    nc.compile()
    return nc


_GRAPH = None


def _get_graph():
    global _GRAPH
    if _GRAPH is None:
        _GRAPH = build_graph()
    return _GRAPH


_PERM = ((0, 3), (1, 2))


def _gtile(t, h):
    return 4 * (t // 2) + _PERM[h][t % 2]


def _make_amask(h):
    am = np.zeros((128, 1024), np.float32)
    r = np.arange(128)[:, None]
    c = np.arange(512)[None, :]
    for j in (0, 1):
        m = _PERM[h][j]
        am[:, j * 512:(j + 1) * 512] = np.where(c <= m * 128 + r, 0.0, NEG)
    return am


def _prep_in_maps(queries, keys, values, Wq, bq, Wk, bk, Wv, bv):
    qT = [np.ascontiguousarray(queries[b].T).astype(BF16) for b in range(B)]
    kT = [np.ascontiguousarray(keys[b].T).astype(BF16) for b in range(B)]
    vT = [np.ascontiguousarray(values[b].T).astype(BF16) for b in range(B)]
    wq_t = np.ascontiguousarray(Wq.T).astype(BF16)
    wk_t = np.ascontiguousarray(Wk.T).astype(BF16)
    wv_t = np.ascontiguousarray(Wv.T).astype(BF16)
    bq_r = np.ascontiguousarray(bq.reshape(4, 128).T).astype(np.float32)
    bk_r = np.ascontiguousarray(bk.reshape(4, 128).T).astype(np.float32)
    bv_r = np.ascontiguousarray(np.broadcast_to(bv.astype(np.float32), (128, H)))

    in_maps = []
    for core in range(NCORES):
        b, h = core // 2, core % 2
        cols = np.concatenate(
            [np.arange(128 * _gtile(t, h), 128 * _gtile(t, h) + 128) for t in range(NT)]
        )
        in_maps.append({
            "xq": np.ascontiguousarray(qT[b][:, cols]),
            "xk": kT[b],
            "xv": vT[b],
            "wq": wq_t, "wk": wk_t, "wv": wv_t,
            "bq": bq_r, "bk": bk_r, "bv": bv_r,
            "amask": _make_amask(h),
        })
    return in_maps


def run(queries, keys, values, Wq, bq, Wk, bk, Wv, bv, trace=False, **spmd_kwargs):
    nc = _get_graph()
    in_maps = _prep_in_maps(
        np.asarray(queries, np.float32), np.asarray(keys, np.float32),
        np.asarray(values, np.float32), np.asarray(Wq, np.float32),
        np.asarray(bq, np.float32), np.asarray(Wk, np.float32),
        np.asarray(bk, np.float32), np.asarray(Wv, np.float32),
        np.asarray(bv, np.float32),
    )
    res = run_bass_kernel_spmd(
        nc, in_maps, core_ids=list(range(NCORES)), trace=trace, **spmd_kwargs
    )
    context = np.empty((B, S, H), np.float32)
    attn = np.empty((B, S, S), np.float32)
    for core in range(NCORES):
        r = res.results[core]
        b, h = core // 2, core % 2
        for t in range(NT):
            g = _gtile(t, h)
            attn[b, 128 * g:128 * g + 128, :] = r["attn"][128 * t:128 * t + 128, :]
            context[b, 128 * g:128 * g + 128, :] = r["ctx"][128 * t:128 * t + 128, :]
    return (context, attn), res


def kernel(queries, keys, values, Wq, bq, Wk, bk, Wv, bv):
    (context, attn), _ = run(queries, keys, values, Wq, bq, Wk, bk, Wv, bv)
    return context, attn


# revision 35
# speedup vs baseline: 1.0420x; 1.0420x over previous
"""Causal scaled-dot attention (with QKV projections) on 8 TRN2 NeuronCores.

Sharding: data-parallel over (batch, query-half).  core = (b = c//2, h = c%2).
Each core owns 16 q-tiles of 128 rows of one batch, interleaved so that every
core sees the identical causal structure: local tile t maps to global tile
g = 4*(t//2) + 2*h + (t%2), giving W(t) = t//2 + 1 key-chunks of 512 for all
cores (SPMD-uniform graph), and the within-diagonal mask offset m = g%4 =
2*h + (t%2) is supplied as per-core mask *data*.

Per-core on-chip pipeline (all matmuls bf16, f32 PSUM accumulation):
  qT/kT projections in transposed layout (bias per-partition on ACT),
  v projection in natural layout (bias via a K=1 ones-matmul row),
  scores = qT.T @ kT  -> additive -1e9 causal mask on the diagonal chunk ->
  exp on ACT (scale=1/sqrt(H), row-sums via accum_out) ->
  xbar DMA-transpose of the bf16 exp tiles -> AV matmul accumulated in PSUM ->
  context scaled by 1/rowsum;  attention output = exp * 1/rowsum (f32).
The strictly-masked key-chunks are never computed nor written: output DRAM is
pre-zeroed by the runtime (run_bass_kernel_spmd contract).
"""

import os
import sys

sys.path.insert(0, "/opt/trn_rl_repo")
os.environ.setdefault("MYCRO_LOCAL_CACHE", "1")
os.environ.setdefault("JAX_PLATFORMS", "")

import numpy as np
import ml_dtypes

import concourse.bass as bass  # noqa: F401  (engine types referenced via nc)
import concourse.mybir as mybir
import concourse.tile as tile
from concourse import bacc
from concourse.bass_utils import run_bass_kernel_spmd

BF16 = ml_dtypes.bfloat16
B, S, H = 4, 4096, 512
NCORES = 8
ROWS = S // 2          # 2048 query rows per core
NT = ROWS // 128       # 16 q-tiles per core
SCALE = 1.0 / float(np.sqrt(H))
NEG = -1.0e9

F32 = mybir.dt.float32
BF = mybir.dt.bfloat16


def _build_body(ctx, tc, p):
    nc = tc.nc
    Exp = mybir.ActivationFunctionType.Exp
    Ident = mybir.ActivationFunctionType.Identity
    Copy = mybir.ActivationFunctionType.Copy
    AX = mybir.AxisListType.X

    consts = ctx.enter_context(tc.tile_pool(name="consts", bufs=1))
    resident = ctx.enter_context(tc.tile_pool(name="resident", bufs=1))
    stage = ctx.enter_context(tc.tile_pool(name="stage", bufs=8))
    pspool = ctx.enter_context(tc.tile_pool(name="ps", bufs=5, space="PSUM"))
    psctx = ctx.enter_context(tc.tile_pool(name="psctx", bufs=3, space="PSUM"))
    work = ctx.enter_context(tc.tile_pool(name="work", bufs=2))
    ptp = ctx.enter_context(tc.tile_pool(name="ptp", bufs=2))

    ldst = nc.scalar      # HWDGE ring for loads + stores (ACT sequencer)
    tring = nc.sync       # HWDGE ring dedicated to xbar transposes (SP sequencer)

    # ---- constants (SWDGE; small) ---------------------------------------
    w_sb = {}
    for wname in ("wq", "wk", "wv"):
        tiles = [consts.tile([128, H], BF, tag=f"{wname}{c}", name=f"{wname}{c}")
                 for c in range(4)]
        w_sb[wname] = tiles
    bq_sb = consts.tile([128, 4], F32, tag="bq")
    bk_sb = consts.tile([128, 4], F32, tag="bk")
    bv_sb = consts.tile([128, H], F32, tag="bvf")
    am_sb = consts.tile([128, 1024], F32, tag="am")
    ident = consts.tile([128, 128], BF, tag="ident")
    for c in range(4):
        ldst.dma_start(out=w_sb["wq"][c][:], in_=p["wq"][c * 128:(c + 1) * 128, :])
    nc.gpsimd.dma_start(out=bq_sb[:], in_=p["bq"][:, :])
    nc.gpsimd.dma_start(out=am_sb[:], in_=p["amask"][:, :])
    for c in range(4):
        nc.gpsimd.dma_start(out=w_sb["wk"][c][:], in_=p["wk"][c * 128:(c + 1) * 128, :])
    nc.gpsimd.dma_start(out=bk_sb[:], in_=p["bk"][:, :])
    for c in range(4):
        nc.gpsimd.dma_start(out=w_sb["wv"][c][:], in_=p["wv"][c * 128:(c + 1) * 128, :])
    nc.gpsimd.dma_start(out=bv_sb[:], in_=p["bv"][:, :])
    from concourse.masks import make_identity
    make_identity(nc, ident[:])

    # ---- resident tensors and quarter-granular staging ------------------
    qT = [resident.tile([128, ROWS], BF, tag=f"qT{c}", name=f"qT{c}") for c in range(4)]
    kT = [resident.tile([128, S], BF, tag=f"kT{c}", name=f"kT{c}") for c in range(4)]
    v_sb = [resident.tile([128, 512], BF, tag=f"v{i}", name=f"v{i}") for i in range(32)]

    xq_q = {}   # quarter -> 4 tiles [128, 512]
    xk_q = {}   # quarter -> 4 tiles [128, 1024]
    xv_q = {}   # quarter -> 4 tiles [128, 1024]

    def load_xq(q):
        eng = ldst
        ts_ = [stage.tile([128, 512], BF, tag="stgq", name="stgq", bufs=4) for _ in range(4)]
        for c in range(4):
            eng.dma_start(out=ts_[c][:],
                          in_=p["xq"][c * 128:(c + 1) * 128, q * 512:(q + 1) * 512])
        xq_q[q] = ts_

    def load_xk(q):
        eng = ldst
        ts_ = [stage.tile([128, 1024], BF, tag="stgk", name="stgk") for _ in range(4)]
        for c in range(4):
            eng.dma_start(out=ts_[c][:],
                          in_=p["xk"][c * 128:(c + 1) * 128, q * 1024:(q + 1) * 1024])
        xk_q[q] = ts_

    def load_xv(q):
        ts_ = [stage.tile([128, 1024], BF, tag="stgv", name="stgv") for _ in range(4)]
        for c in range(4):
            ldst.dma_start(out=ts_[c][:],
                           in_=p["xv"][c * 128:(c + 1) * 128, q * 1024:(q + 1) * 1024])
        xv_q[q] = ts_

    def proj_qT(icc):
        for hc in range(4):
            ps = pspool.tile([128, 512], F32, tag="ps")
            for pc in range(4):
                nc.tensor.matmul(
                    ps[:], lhsT=w_sb["wq"][pc][:, hc * 128:(hc + 1) * 128],
                    rhs=xq_q[icc][pc][:], start=(pc == 0), stop=(pc == 3),
                )
            nc.vector.tensor_scalar_add(
                qT[hc][:, icc * 512:(icc + 1) * 512], ps[:], bq_sb[:, hc:hc + 1]
            )

    def proj_kT(sc):
        q, off = sc // 2, (sc % 2) * 512
        for hc in range(4):
            ps = pspool.tile([128, 512], F32, tag="ps")
            for pc in range(4):
                nc.tensor.matmul(
                    ps[:], lhsT=w_sb["wk"][pc][:, hc * 128:(hc + 1) * 128],
                    rhs=xk_q[q][pc][:, off:off + 512], start=(pc == 0), stop=(pc == 3),
                )
            nc.vector.tensor_scalar_add(
                kT[hc][:, sc * 512:(sc + 1) * 512], ps[:], bk_sb[:, hc:hc + 1]
            )

    def proj_v(sc):
        q, off = sc // 8, (sc % 8) * 128
        ps = pspool.tile([128, 512], F32, tag="ps")
        for pc in range(4):
            nc.tensor.matmul(
                ps[:], lhsT=xv_q[q][pc][:, off:off + 128], rhs=w_sb["wv"][pc][:],
                start=(pc == 0), stop=(pc == 3),
            )
        nc.vector.tensor_copy(v_sb[sc][:], ps[:])

    # incremental emission state
    done_qT = [False] * 4
    done_kT = [False] * 8
    done_v = [False] * 4     # per xv quarter (8 v tiles at a time)

    def need_qT(icc):
        if not done_qT[icc]:
            load_xq(icc)
            proj_qT(icc)
            done_qT[icc] = True

    def need_kT(sc):
        if not done_kT[sc]:
            if sc % 2 == 0 and sc // 2 not in xk_q:
                load_xk(sc // 2)
            proj_kT(sc)
            done_kT[sc] = True

    def need_v(q):
        if not done_v[q]:
            load_xv(q)
            for sc in range(8 * q, 8 * q + 8):
                proj_v(sc)
            done_v[q] = True

    # prologue: enough for tiles 0..1
    need_qT(0)
    need_kT(0)
    need_v(0)

    # ---- attention main loop (AV/output stage software-pipelined by 1) ---
    tile_state = {}

    def emit_scores(t):
        W = t // 2 + 1           # number of 512-wide key chunks
        j = t % 2                # mask slice; true offset m is in the mask data
        nd = 2 + 2 * j           # diagonal-chunk 128-subchunks worth computing
        dw = nd * 128            # diagonal-chunk computed width (cols beyond are masked)

        ebf = work.tile([128, S], BF, tag="ebf", bufs=4)
        lc = work.tile([128, 8], F32, tag="lc", bufs=3)
        ch1 = (W + 1) // 2
        pts = []
        for kb in range(W):
            cw = 512 if kb < W - 1 else dw
            ps = pspool.tile([128, 512], F32, tag="ps")
            for pc in range(4):
                nc.tensor.matmul(
                    ps[:, 0:cw], lhsT=qT[pc][:, t * 128:(t + 1) * 128],
                    rhs=kT[pc][:, kb * 512:kb * 512 + cw],
                    start=(pc == 0), stop=(pc == 3),
                )
            if kb == W - 1:
                nc.vector.tensor_add(ps[:, 0:cw], ps[:, 0:cw],
                                     am_sb[:, j * 512:j * 512 + cw])
            nc.scalar.activation(
                out=ebf[:, kb * 512:kb * 512 + cw], in_=ps[:, 0:cw],
                func=Exp, scale=SCALE, accum_out=lc[:, kb:kb + 1],
            )
            # issue each half-strip xbar transpose as soon as its exps are done
            for (lo, hi) in ((0, ch1), (ch1, W)):
                if hi > lo and kb == hi - 1:
                    w_cols = (hi - lo - 1) * 512 + (512 if hi < W else dw)
                    pt = ptp.tile([128, 2048], BF, tag="pt", bufs=6)
                    if W == 1:
                        # tiny final tiles: transpose on the PE itself so the
                        # AV matmuls never wait on a DMA semaphore
                        pst = pspool.tile([128, 512], BF, tag="ps", name="pst")
                        for c in range(nd):
                            nc.tensor.transpose(
                                pst[:, c * 128:(c + 1) * 128],
                                ebf[:, c * 128:(c + 1) * 128], ident[:],
                            )
                        nc.scalar.activation(
                            out=pt[:, 0:w_cols], in_=pst[:, 0:w_cols], func=Copy)
                    else:
                        tring.dma_start_transpose(
                            out=pt[:, 0:w_cols].rearrange(
                                "q (c f) -> q c f", f=128),
                            in_=ebf[:, lo * 512:lo * 512 + w_cols],
                        )
                    pts.append((lo, hi, pt))
        tile_state[t] = (ebf, lc, pts)

    def emit_outputs(t):
        W = t // 2 + 1
        j = t % 2
        nd = 2 + 2 * j
        nmm = (W - 1) * 4 + nd
        ebf, lc, pts = tile_state.pop(t)

        ps_ctx = psctx.tile([128, 512], F32, tag="psctx")
        k = 0
        for (lo, hi, pt) in pts:
            for kb in range(lo, hi):
                nsub = 4 if kb < W - 1 else nd
                for c in range(nsub):
                    cc = kb * 4 + c
                    nc.tensor.matmul(
                        ps_ctx[:],
                        lhsT=pt[:, (cc - lo * 4) * 128:(cc - lo * 4 + 1) * 128],
                        rhs=v_sb[cc][:], start=(k == 0), stop=(k == nmm - 1),
                    )
                    k += 1

        ls = work.tile([128, 1], F32, tag="ls")
        nc.vector.reduce_sum(ls[:], lc[:, 0:W], axis=AX)
        rr = work.tile([128, 1], F32, tag="rr")
        nc.vector.reciprocal(rr[:], ls[:])

        tw = (W - 1) * 512 + nd * 128
        halves = [(0, min((W + 1) // 2 * 512, tw)), ((W + 1) // 2 * 512, tw)]
        for lo, hi in halves:
            if hi <= lo:
                continue
            outf = work.tile([128, 2048], BF, tag="outf", bufs=3)
            nc.vector.tensor_scalar_mul(outf[:, 0:hi - lo], ebf[:, lo:hi], rr[:])
            nc.gpsimd.dma_start(
                out=p["attn"][t * 128:(t + 1) * 128, lo:hi],
                in_=outf[:, 0:hi - lo],
            )
        cs = work.tile([128, 512], F32, tag="cs")
        nc.scalar.activation(out=cs[:], in_=ps_ctx[:], func=Copy, scale=rr[:])
        nc.vector.tensor_add(cs[:], cs[:], bv_sb[:])
        nc.gpsimd.dma_start(out=p["ctx"][t * 128:(t + 1) * 128, :], in_=cs[:])

    order = list(range(2, NT)) + [0, 1]
    for pos, t in enumerate(order):
        W = t // 2 + 1
        nd = 2 + 2 * (t % 2)
        need_qT(t // 4)
        for sc in range(W):
            need_kT(sc)
        for q in range((4 * (W - 1) + nd + 7) // 8):
            need_v(q)
        if pos + 2 < NT:
            tn = order[pos + 2]
            need_qT(tn // 4)
            need_kT(tn // 2)
            need_v(min((4 * (tn // 2) + 4 + 7) // 8, 3))
        if pos > 1:
            emit_outputs(order[pos - 2])
        emit_scores(t)
    emit_outputs(order[-2])
    emit_outputs(order[-1])


def build_graph():
    nc = bacc.Bacc("TRN2", target_bir_lowering=False, debug=False, num_devices=NCORES)
    p = {}
    p["xq"] = nc.declare_dram_parameter("xq", [H, ROWS], BF, isOutput=False)
    p["xk"] = nc.declare_dram_parameter("xk", [H, S], BF, isOutput=False)
    p["xv"] = nc.declare_dram_parameter("xv", [H, S], BF, isOutput=False)
    for w in ("wq", "wk", "wv"):
        p[w] = nc.declare_dram_parameter(w, [H, H], BF, isOutput=False)
    p["bq"] = nc.declare_dram_parameter("bq", [128, 4], F32, isOutput=False)
    p["bk"] = nc.declare_dram_parameter("bk", [128, 4], F32, isOutput=False)
    p["bv"] = nc.declare_dram_parameter("bv", [128, H], F32, isOutput=False)
    p["amask"] = nc.declare_dram_parameter("amask", [128, 1024], F32, isOutput=False)
    p["attn"] = nc.declare_dram_parameter("attn", [ROWS, S], F32, isOutput=True)
    p["ctx"] = nc.declare_dram_parameter("ctx", [ROWS, H], F32, isOutput=True)

    import contextlib

    with contextlib.ExitStack() as ctx:
        tc = ctx.enter_context(tile.TileContext(nc))
        _build_body(ctx, tc, p)
    nc.compile()
    return nc


_GRAPH = None


def _get_graph():
    global _GRAPH
    if _GRAPH is None:
        _GRAPH = build_graph()
    return _GRAPH


_PERM = ((0, 3), (1, 2))


def _gtile(t, h):
    return 4 * (t // 2) + _PERM[h][t % 2]


def _make_amask(h):
    am = np.zeros((128, 1024), np.float32)
    r = np.arange(128)[:, None]
    c = np.arange(512)[None, :]
    for j in (0, 1):
        m = _PERM[h][j]
        am[:, j * 512:(j + 1) * 512] = np.where(c <= m * 128 + r, 0.0, NEG)
    return am


def _prep_in_maps(queries, keys, values, Wq, bq, Wk, bk, Wv, bv):
    qT = [np.ascontiguousarray(queries[b].T).astype(BF16) for b in range(B)]
    kT = [np.ascontiguousarray(keys[b].T).astype(BF16) for b in range(B)]
    vT = [np.ascontiguousarray(values[b].T).astype(BF16) for b in range(B)]
    wq_t = np.ascontiguousarray(Wq.T).astype(BF16)
    wk_t = np.ascontiguousarray(Wk.T).astype(BF16)
    wv_t = np.ascontiguousarray(Wv.T).astype(BF16)
    bq_r = np.ascontiguousarray(bq.reshape(4, 128).T).astype(np.float32)
    bk_r = np.ascontiguousarray(bk.reshape(4, 128).T).astype(np.float32)
    bv_r = np.ascontiguousarray(np.broadcast_to(bv.astype(np.float32), (128, H)))

    in_maps = []
    for core in range(NCORES):
        b, h = core // 2, core % 2
        cols = np.concatenate(
            [np.arange(128 * _gtile(t, h), 128 * _gtile(t, h) + 128) for t in range(NT)]
        )
        in_maps.append({
            "xq": np.ascontiguousarray(qT[b][:, cols]),
            "xk": kT[b],
            "xv": vT[b],
            "wq": wq_t, "wk": wk_t, "wv": wv_t,
            "bq": bq_r, "bk": bk_r, "bv": bv_r,
            "amask": _make_amask(h),
        })
    return in_maps


def run(queries, keys, values, Wq, bq, Wk, bk, Wv, bv, trace=False, **spmd_kwargs):
    nc = _get_graph()
    in_maps = _prep_in_maps(
        np.asarray(queries, np.float32), np.asarray(keys, np.float32),
        np.asarray(values, np.float32), np.asarray(Wq, np.float32),
        np.asarray(bq, np.float32), np.asarray(Wk, np.float32),
        np.asarray(bk, np.float32), np.asarray(Wv, np.float32),
        np.asarray(bv, np.float32),
    )
    res = run_bass_kernel_spmd(
        nc, in_maps, core_ids=list(range(NCORES)), trace=trace, **spmd_kwargs
    )
    context = np.empty((B, S, H), np.float32)
    attn = np.empty((B, S, S), np.float32)
    for core in range(NCORES):
        r = res.results[core]
        b, h = core // 2, core % 2
        for t in range(NT):
            g = _gtile(t, h)
            attn[b, 128 * g:128 * g + 128, :] = r["attn"][128 * t:128 * t + 128, :]
            context[b, 128 * g:128 * g + 128, :] = r["ctx"][128 * t:128 * t + 128, :]
    return (context, attn), res


def kernel(queries, keys, values, Wq, bq, Wk, bk, Wv, bv):
    (context, attn), _ = run(queries, keys, values, Wq, bq, Wk, bk, Wv, bv)
    return context, attn
